# revision 8
# baseline (speedup 1.0000x reference)
"""Trainium2 Bass kernel for AsymmetricWeightsDequantizer.

result = zero_point + weight * scale  (per [O, G] group, broadcast over GS)
         + svd_up @ svd_down          (rank-128 correction)

Sharding: output dim O split across 8 cores (1024 rows each), svd_down
replicated.

v4 structure (wide ops only; engines balanced):
  per 2048-col block of each 128-row tile:
   - DVE (or GPSIMD for one block per tile): ONE wide paged-broadcast
         tensor_tensor  q[p,(g,j)] = w[p,(g,j)] * scale[p,g]
         (scale AP has 0-stride over the 128 in-group cols -> 16 groups
         in one instruction)
   - PE:  psum = [up | z_hi | z_lo*16] @ [down | E | E/16]  -- one fp8e4
          DoubleRow matmul per 512 cols (contract 256 at 2 MACs/cell),
          then psum += I @ q (fp16 identity accumulate)
   - ACT: ONE wide activation(Copy) psum -> out_sb fp16
   - two blocks skip id+ACT entirely: DVE adds q+psum -> out fp16 (TT2)
   - DMA: out written as fp16 (host upcasts to fp32); halves write traffic
"""

import numpy as np
import ml_dtypes

import concourse.bass as bass
import concourse.bacc as bacc
import concourse.mybir as mybir
import concourse.tile as tile
from concourse import bass_utils

O, G, GS = 8192, 64, 128
I = G * GS              # 8192
RANK = 128
NCORES = 8
OP = O // NCORES        # 1024 rows per core
NT = OP // 128          # 8 partition tiles per core
NBLK = 8                # 1024-col blocks per row tile
BLK = I // NBLK         # 1024
GPB = G // NBLK         # 8 groups per block
NPS = BLK // 512        # 512-col DR matmul slices per block

# block roles, by linear index t*NBLK+nb (64 sub-blocks per core):
# ~1/5 of muls on GPSIMD, ~1/11 of blocks take the DVE q+psum path (no
# id matmul / ACT copy) -- balances DVE vs ACT vs PE occupancy
GP_BLOCKS = {(i // NBLK, i % NBLK) for i in range(NT * NBLK) if i % 5 == 2}
TT2_BLOCKS = {(i // NBLK, i % NBLK) for i in range(NT * NBLK)
              if i % 11 == 7 and i % 5 != 2}

BF16 = ml_dtypes.bfloat16
FP8 = ml_dtypes.float8_e4m3fn
F32 = mybir.dt.float32
FP16 = mybir.dt.float16
F8 = mybir.dt.float8e4
U8 = mybir.dt.uint8

_cached_nc = None


def _build():
    global _cached_nc
    if _cached_nc is not None:
        return _cached_nc

    nc = bacc.Bacc("TRN2", target_bir_lowering=False, debug=False,
                   num_devices=NCORES)

    w_d = nc.dram_tensor("w", [OP, I], U8, kind="ExternalInput")
    sc_d = nc.dram_tensor("scale_r", [128, NT * G], F32, kind="ExternalInput")
    # stationary planes: [up | zp_hi/lo stack], fp8e4, channel-plane layout
    st_d = nc.dram_tensor("stat", [128, 2 * OP], F8, kind="ExternalInput")
    # moving planes: [down | E/E-over-16 stack]
    cb_d = nc.dram_tensor("comb", [128, 2 * I], F8, kind="ExternalInput")
    id_d = nc.dram_tensor("ident", [128, 128], FP16, kind="ExternalInput")
    out_d = nc.dram_tensor("out", [OP, I], FP16, kind="ExternalOutput")

    with tile.TileContext(nc) as tc:
        with (
            tc.tile_pool(name="const", bufs=1) as cpool,
            tc.tile_pool(name="wp", bufs=3) as wpool,
            tc.tile_pool(name="qp", bufs=6) as qpool,
            tc.tile_pool(name="outp", bufs=2) as opool,
            tc.tile_pool(name="ps", bufs=4, space="PSUM") as pspool,
        ):
            st_sb = cpool.tile([128, 2 * OP], F8)
            cb_sb = cpool.tile([128, 2 * I], F8)
            sc_sb = cpool.tile([128, NT * G], F32)
            id_sb = cpool.tile([128, 128], FP16)

            st3 = st_sb[:].rearrange("p (c m) -> p c m", c=2)
            cb3 = cb_sb[:].rearrange("p (c n) -> p c n", c=2)
            cb3_d = cb_d[:].rearrange("p (c n) -> p c n", c=2)

            # small consts + first comb chunk first so compute starts early
            nc.sync.dma_start(sc_sb[:], sc_d[:])
            nc.sync.dma_start(id_sb[:], id_d[:])
            nc.sync.dma_start(st_sb[:], st_d[:])
            nc.sync.dma_start(cb3[:, :, 0:2048], cb3_d[:, :, 0:2048])

            for t in range(NT):
                w_sb = wpool.tile([128, I], U8)
                nc.sync.dma_start(w_sb[:], w_d[t * 128:(t + 1) * 128, :])
                if t == 0:
                    for j in range(1, 4):
                        nc.sync.dma_start(cb3[:, :, j * 2048:(j + 1) * 2048],
                                          cb3_d[:, :, j * 2048:(j + 1) * 2048])
                out_sb = opool.tile([128, I], FP16)

                for nb in range(NBLK):
                    ps = pspool.tile([128, BLK], F32)
                    q = qpool.tile([128, BLK], FP16)
                    is_tt2 = (t, nb) in TT2_BLOCKS

                    # paged mul: q = w * scale (one wide op per block)
                    w3 = w_sb[:, nb * BLK:(nb + 1) * BLK].rearrange(
                        "p (g j) -> p g j", g=GPB)
                    q3 = q[:].rearrange("p (g j) -> p g j", g=GPB)
                    scb = sc_sb[:, t * G + nb * GPB:
                                t * G + (nb + 1) * GPB].unsqueeze(2)
                    eng = nc.gpsimd if (t, nb) in GP_BLOCKS else nc.vector
                    eng.tensor_tensor(
                        q3, w3, scb.broadcast_to((128, GPB, GS)),
                        op=mybir.AluOpType.mult)

                    # PE: fused svd+zp DoubleRow fp8 matmuls (contract 256)
                    for k in range(NPS):
                        n = nb * NPS + k
                        nc.tensor.matmul(
                            ps[:, k * 512:(k + 1) * 512],
                            st3[:, :, t * 128:(t + 1) * 128],
                            cb3[:, :, n * 512:(n + 1) * 512],
                            start=True, stop=(is_tt2 and k == NPS - 1),
                            perf_mode=mybir.MatmulPerfMode.DoubleRow,
                        )
                    if is_tt2:
                        # DVE: out = q + psum directly (fp16), no id/ACT
                        nc.vector.tensor_tensor(
                            out_sb[:, nb * BLK:(nb + 1) * BLK], q[:], ps[:],
                            op=mybir.AluOpType.add)
                    else:
                        # PE: identity accumulate of q (fp16 moving max 512)
                        for k in range(NPS):
                            nc.tensor.matmul(
                                ps[:, k * 512:(k + 1) * 512],
                                id_sb[:],
                                q[:, k * 512:(k + 1) * 512],
                                start=False, stop=True,
                            )
                        # ACT: one wide copy psum -> fp16 out subtile
                        nc.scalar.activation(
                            out_sb[:, nb * BLK:(nb + 1) * BLK], ps[:],
                            mybir.ActivationFunctionType.Copy,
                            bias=0.0, scale=1.0)

                nc.sync.dma_start(out_d[t * 128:(t + 1) * 128, :], out_sb[:])

    nc.compile()
    _cached_nc = nc
    return nc


def _make_in_maps(weight, scale, zero_point, svd_up, svd_down):
    w = np.ascontiguousarray(weight.reshape(O, I)).astype(np.uint8)
    sc = np.ascontiguousarray(scale.reshape(O, G).astype(np.float32))
    zp = np.ascontiguousarray(zero_point.reshape(O, G).astype(np.float32))
    down8 = np.ascontiguousarray(svd_down).astype(FP8)       # [RANK, I]

    # group indicator planes: rows 0..63 = E (for zp_hi), 64..127 = E/16
    # (the lo channel is pre-scaled x16 so values stay in fp8 normal range)
    eb2 = np.zeros((128, I), dtype=np.float32)
    for g in range(G):
        eb2[g, g * GS:(g + 1) * GS] = 1.0
        eb2[G + g, g * GS:(g + 1) * GS] = 1.0 / 16.0
    comb = np.concatenate([down8.astype(np.float32), eb2],
                          axis=1).astype(FP8)                # [128, 2I]

    ident = np.eye(128, dtype=np.float16)

    in_maps = []
    for c in range(NCORES):
        sl = slice(c * OP, (c + 1) * OP)
        scr = np.ascontiguousarray(
            sc[sl].reshape(NT, 128, G).transpose(1, 0, 2).reshape(
                128, NT * G))
        z = zp[sl]                           # [OP, G] f32
        z_hi = z.astype(FP8)
        z_lo = ((z - z_hi.astype(np.float32)) * 16.0).astype(FP8)
        zeroT2 = np.concatenate([z_hi.T, z_lo.T], axis=0)    # [128, OP] fp8
        upT8 = np.ascontiguousarray(svd_up[sl].T).astype(FP8)
        stat = np.concatenate([upT8.astype(np.float32),
                               zeroT2.astype(np.float32)],
                              axis=1).astype(FP8)            # [128, 2*OP]
        in_maps.append({
            "w": np.ascontiguousarray(w[sl]),
            "scale_r": scr,
            "stat": np.ascontiguousarray(stat),
            "comb": np.ascontiguousarray(comb),
            "ident": ident,
        })
    return in_maps


def _run(in_maps, trace=False, **kwargs):
    nc = _build()
    return bass_utils.run_bass_kernel_spmd(
        nc, in_maps, core_ids=list(range(NCORES)), trace=trace, **kwargs)


def kernel(weight, scale, zero_point, svd_up, svd_down):
    in_maps = _make_in_maps(np.asarray(weight), np.asarray(scale),
                            np.asarray(zero_point), np.asarray(svd_up),
                            np.asarray(svd_down))
    res = _run(in_maps)
    out = np.concatenate([res.results[c]["out"] for c in range(NCORES)],
                         axis=0)
    return out.astype(np.float32)
